# revision 52
# baseline (speedup 1.0000x reference)
"""Causal self-attention (B=4, T=2048, C=1024, 16 heads) on 8 trn2 NeuronCores.

Sharding: batch x head-group hybrid. Core c handles batch c//2 and head
group c%2 (8 of 16 heads). Each core computes the qkv projection for its
head group over its batch's tokens, runs causal attention for its 8
heads, and produces a partial c_proj output (contraction over its 512 of
the 1024 y channels). Host sums the two partials per batch and adds the
constant row b_proj + b_v @ W_proj (exact; see shard_inputs docstring).

Structure (v6):
  x^T is prepared on the HOST (transpose + bf16) and DMA'd directly: no
  PE transposes. All large host tensors are pre-shuffled to partition-
  major layouts so each resident SBUF tensor is ONE tile filled by ONE
  (or a few just-in-time) DMAs -- the HWDGE charges ~625ns of serialized
  overhead per dma_start, so DMA COUNT matters more than bytes.
    Q^T [j, tok]      = W_q^T x^T  (bf16 matmuls; + b_q in the copy),
                        stored fp8 twice (planes 0/1 duplicated by one
                        SBUF->SBUF DMA per chunk -- stride-0 moving APs
                        fail walrus codegen).
    K~  [j, tok]      = (log2e/8 W_k)^T x^T, stored as fp8 (hi, lo)
                        planes: hi + lo carries ~bf16 precision.
    V'  [tok, 65]     = x W_v bf16 (col 64 = ones so P@V' also emits
                        softmax denominators).
    S^T [k_tok, q]    = fp8 DoubleRow matmuls: stationary K~ (hi,lo),
                        moving Q8 (dup planes), 2 rows/cycle -- half the
                        bf16 stream cost, only Q8 rounding as extra
                        error (measured 1.37e-2 vs the 2e-2 gate; any
                        further fp8 -- x/W/P/V/y -- fails the gate).
    P = 2^(S^T)       bf16 ScalarE Exp(scale=ln2); causal diagonal
                        blocks masked by a triu multiply on the otherwise
                        idle GpSimd engine (SBUF-only op, off the
                        exp->PV chain).
    O' [65, q]        = V'.T P accumulated over k tiles (bf16).
    y [128, tok]      per head pair, normalized via ones-column
                        denominators: per chunk ONE strided DMA gathers
                        all 4 pairs' denominator rows into [2,4,512],
                        one reciprocal + f32r cast, then per-pair f32r
                        selector-broadcast matmuls + y multiply. Head B
                        rows cross partitions via one SBUF DMA per chunk.
    out [tok, C]      = y^T W_proj, both 512-col halves staged into one
                        [128,1024] tile, one DMA per token tile.

Scheduling: two deferred queues keep PE (bottleneck, ~200us busy) fed.
Priority queue: next chunk's projection pieces + normalization finishes
(emission-order barrier before each chunk's attention). Late queue:
output-projection pieces -- drained only when the priority queue is
empty, which parks them in the late chunks where attention alone leaves
PE short of work (exp on ScalarE is the per-kt critical path there).
Measured end-to-end relative error vs the fp32 reference: ~1.4e-2.
"""

from contextlib import ExitStack

import numpy as np
import ml_dtypes

import concourse.bass as bass
import concourse.mybir as mybir
import concourse.tile as tile
from concourse import bacc
from concourse.bass_utils import run_bass_kernel_spmd

F32 = mybir.dt.float32
BF16 = mybir.dt.bfloat16
F8 = mybir.dt.float8e4
DR = mybir.MatmulPerfMode.DoubleRow
LN2 = float(np.log(2.0))
K_SCALE = float(0.125 * np.log2(np.e))  # folded into W_k on the host

T = 2048
C = 1024
NH_LOC = 8          # heads per core
HD = 64
J = NH_LOC * HD     # 512 local q/k/v channels
N_CORES = 8
QC = 4              # q chunks of 512
TOK_TILES = 16      # token tiles of 128
C_TILES = 8         # contraction tiles of 128 over C
PAIRS = 4           # head pairs per core


def build_nc(debug_taps=False):
    nc = bacc.Bacc("TRN2", target_bir_lowering=False, debug=False)
    dbg = {}
    if debug_taps:
        dbg["y"] = nc.dram_tensor("dbg_y", [128, PAIRS, T], BF16,
                                  kind="ExternalOutput")
        dbg["qt"] = nc.dram_tensor("dbg_qt", [128, PAIRS, 2, T], F8,
                                   kind="ExternalOutput")
        dbg["kt"] = nc.dram_tensor("dbg_kt", [128, PAIRS, 2, T], F8,
                                   kind="ExternalOutput")
        dbg["v"] = nc.dram_tensor("dbg_v", [128, TOK_TILES * NH_LOC * 65],
                                  BF16, kind="ExternalOutput")

    # host-pre-shuffled partition-major layouts (see shard_inputs)
    xt_d = nc.dram_tensor("xt", [128, C_TILES, T], BF16, kind="ExternalInput")
    wqk_d = nc.dram_tensor("wqk", [128, 8, C_TILES, 128], BF16,
                           kind="ExternalInput")
    bq_d = nc.dram_tensor("bq", [J], F32, kind="ExternalInput")
    wv_d = nc.dram_tensor("wv", [128, C_TILES, J], BF16, kind="ExternalInput")
    wp_d = nc.dram_tensor("wp", [128, PAIRS, C], BF16, kind="ExternalInput")
    out_d = nc.dram_tensor("out", [T, C], BF16, kind="ExternalOutput")

    with tile.TileContext(nc) as tc, ExitStack() as ctx:
        const = ctx.enter_context(tc.tile_pool(name="const", bufs=1))
        wpool = ctx.enter_context(tc.tile_pool(name="w", bufs=1))
        qkv = ctx.enter_context(tc.tile_pool(name="qkv", bufs=1))
        ypool = ctx.enter_context(tc.tile_pool(name="y", bufs=1))
        wk = ctx.enter_context(tc.tile_pool(name="wk", bufs=1))

        # ---- constants ----
        # triu2[p, c, f] = 1 iff f >= p, duplicated over c: masks the causal
        # diagonal 128-block of both heads' P in one tensor_tensor op.
        triu2 = const.tile([128, 2, 128], BF16)
        nc.gpsimd.memset(triu2, 0.0)
        nc.gpsimd.affine_select(
            out=triu2, in_=triu2, compare_op=mybir.AluOpType.is_gt,
            fill=1.0, base=0, pattern=[[0, 2], [-1, 128]],
            channel_multiplier=1)
        # selab[p, f] = 1 iff f in [64p, 64p+64): head selector for the
        # reciprocal broadcast matmul (partition-1 memsets are illegal).
        selab = const.tile([2, 128], F32)
        nc.gpsimd.memset(selab, 1.0)
        nc.gpsimd.affine_select(
            out=selab, in_=selab, compare_op=mybir.AluOpType.is_ge,
            fill=0.0, base=0, pattern=[[1, 128]], channel_multiplier=-64)
        nc.gpsimd.affine_select(
            out=selab, in_=selab, compare_op=mybir.AluOpType.is_ge,
            fill=0.0, base=63, pattern=[[-1, 128]], channel_multiplier=64)
        selab_r = const.tile([2, 128], mybir.dt.float32r)
        nc.vector.tensor_copy(selab_r, selab)
        bq_sb = const.tile([128, 4], F32)

        # ---- resident weights (one tile each); x^T rotates per chunk ----
        wqk_sb = wpool.tile([128, 8, C_TILES, 128], BF16, name="wqk")
        wv_sb = wpool.tile([128, C_TILES, J], BF16, name="wv")
        wp_sb = wpool.tile([128, PAIRS, C], BF16, name="wp")

        def emit_xt_dma(qc, sliced=False):
            xt = wk.tile([128, C_TILES, 512], BF16, tag="xt", bufs=2,
                         name=f"xt{qc}")
            w0 = qc * 512
            if sliced:  # chunk 0: two halves so the first matmuls start asap
                nc.sync.dma_start(xt[:, 0:4, :], xt_d[:, 0:4, w0:w0 + 512])
                nc.sync.dma_start(xt[:, 4:8, :], xt_d[:, 4:8, w0:w0 + 512])
            else:
                nc.sync.dma_start(xt, xt_d[:, :, w0:w0 + 512])
            return xt

        # ---- persistent activations ----
        # qt/kt: [128 (2 heads x 64 dims), pair, 2 planes, T] fp8.
        #   qt planes duplicate Q8; kt planes are (hi, lo) of log2e/8*K.
        qt_sb = qkv.tile([128, PAIRS, 2, T], F8, name="qt")
        kt_sb = qkv.tile([128, PAIRS, 2, T], F8, name="kt")
        # v: one tile so each token tile's PSUM->SBUF copy is a single op.
        v_sb = qkv.tile([128, TOK_TILES, NH_LOC, 65], BF16, name="v")
        nc.vector.memset(v_sb[:, :, :, 64:65], 1.0)
        y_sb = ypool.tile([128, PAIRS, T], BF16, name="y")

        # ====== fused pipeline: qkv projection chunks overlap attention ====
        # PSUM (8 banks):
        #   S    [128,1024] x2  S tiles / chunk-0 projection accums  4 banks
        #   O    [65,512]   x2  O' accumulators (o_a, o_b)           2 banks
        #   acc  [128,512]  x2  qkv/c_proj accums + recip bcasts    2 banks
        with tc.tile_pool(name="ps", bufs=1, space="PSUM") as psb:
            pending = []       # priority: projection pieces + fins
            pending_late = []  # c_proj pieces: drained in the last chunk,
            allow_late = [False]  # where attention alone starves PE
            fins_hold = []     # fins wait a chunk so their DMA deps are done

            def flush_one():
                if pending:
                    pending.pop(0)()
                elif pending_late and allow_late[0]:
                    pending_late.pop(0)()

            def flush_pending():
                while pending or pending_late:
                    flush_one()

            def a_pieces(qc, xt, acc_tag="acc", acc_bufs=2):
                """Emit-later closures computing Q/K/V projections for qc.
                Chunk 0 runs before attention starts and borrows the idle
                S banks for deeper pipelining. For chunk 0 the weight DMAs
                are emitted just-in-time inside each piece."""
                pieces = []
                w0 = qc * 512

                def qk_piece(jt, half):
                    def run():
                        if half == 0:
                            pm = psb.tile([128, 512], F32, tag=acc_tag,
                                          bufs=acc_bufs, name="pm")
                            half_pm[jt] = pm
                            for ct in range(4):
                                nc.tensor.matmul(
                                    pm,
                                    wqk_sb[:, jt, ct, :],
                                    xt[:, ct, :],
                                    start=(ct == 0), stop=False)
                            return
                        pm = half_pm.pop(jt)
                        for ct in range(4, C_TILES):
                            nc.tensor.matmul(
                                pm,
                                wqk_sb[:, jt, ct, :],
                                xt[:, ct, :],
                                start=False, stop=(ct == C_TILES - 1))
                        if jt < 4:
                            nc.vector.tensor_scalar(
                                qt_sb[:, jt, 0, w0:w0 + 512], pm,
                                bq_sb[:, jt:jt + 1], None,
                                mybir.AluOpType.add)
                        else:
                            p = jt - 4
                            nc.vector.tensor_copy(
                                kt_sb[:, p, 0, w0:w0 + 512], pm)
                            nc.vector.scalar_tensor_tensor(
                                kt_sb[:, p, 1, w0:w0 + 512], pm, 1.0,
                                kt_sb[:, p, 0, w0:w0 + 512],
                                mybir.AluOpType.mult,
                                mybir.AluOpType.subtract)
                    return run

                def qdup_piece():
                    def run():
                        nc.sync.dma_start(qt_sb[:, :, 1, w0:w0 + 512],
                                          qt_sb[:, :, 0, w0:w0 + 512])
                    return run

                def v_piece(tt, half):
                    def run():
                        tta = qc * 4 + tt
                        if half == 0:
                            pv = psb.tile([128, J], F32, tag=acc_tag,
                                          bufs=acc_bufs, name="pv")
                            half_pm[8 + tt] = pv
                            for ct in range(4):
                                nc.tensor.matmul(
                                    pv,
                                    xt[:, ct, tt * 128:(tt + 1) * 128],
                                    wv_sb[:, ct, :],
                                    start=(ct == 0), stop=False)
                            return
                        pv = half_pm.pop(8 + tt)
                        for ct in range(4, C_TILES):
                            nc.tensor.matmul(
                                pv,
                                xt[:, ct, tt * 128:(tt + 1) * 128],
                                wv_sb[:, ct, :],
                                start=False, stop=(ct == C_TILES - 1))
                        nc.vector.tensor_copy(v_sb[:, tta, :, 0:64], pv)
                    return run

                half_pm = {}
                for jt in range(4):
                    pieces.append(qk_piece(jt, 0))
                    pieces.append(qk_piece(jt, 1))
                pieces.append(qdup_piece())
                for jt in range(4, 8):
                    pieces.append(qk_piece(jt, 0))
                    pieces.append(qk_piece(jt, 1))
                for tt in range(4):
                    pieces.append(v_piece(tt, 0))
                    pieces.append(v_piece(tt, 1))
                return pieces

            proj_ob = {}
            proj_po = {}

            def make_proj_piece(tt, oc, quarter, tag="acc", bufs=2,
                                split_dma=False):
                # quarter of an output-projection token tile (2 of the 4
                # pair-accumulation matmuls): finer PE filler granules
                def proj():
                    if oc == 0 and quarter == 0:
                        proj_ob[tt] = wk.tile([128, C], BF16, tag="ob",
                                              bufs=4, name="ob")
                    if quarter == 0:
                        proj_po[(tt, oc)] = psb.tile(
                            [128, 512], F32, tag=tag, bufs=bufs, name="po")
                    po = proj_po[(tt, oc)]
                    prs = (0, 1) if quarter == 0 else (2, 3)
                    for p in prs:
                        nc.tensor.matmul(
                            po,
                            y_sb[:, p, tt * 128:(tt + 1) * 128],
                            wp_sb[:, p, oc * 512:(oc + 1) * 512],
                            start=(p == 0), stop=(p == PAIRS - 1))
                    if quarter == 1:
                        del proj_po[(tt, oc)]
                        ob = proj_ob[tt]
                        nc.vector.tensor_copy(
                            ob[:, oc * 512:(oc + 1) * 512], po)
                        if split_dma:
                            # tail: per-half DMAs so the drain only waits
                            # for the second, smaller transfer
                            nc.sync.dma_start(
                                out_d[tt * 128:(tt + 1) * 128,
                                      oc * 512:(oc + 1) * 512],
                                ob[:, oc * 512:(oc + 1) * 512])
                            if oc == 1:
                                del proj_ob[tt]
                        elif oc == 1:
                            del proj_ob[tt]
                            nc.sync.dma_start(
                                out_d[tt * 128:(tt + 1) * 128, :], ob)
                return proj

            # chunk 0 runs inline on the still-idle S banks. DMA order is
            # the prologue critical path (the HWDGE serializes ~625ns per
            # DMA): first piece's weights, then x^T halves, then the rest.
            # DMA-free warmup matmuls absorb the ~3us first-DMA latency and
            # ramp the PE clock (p-state) before the real work lands.
            nc.sync.dma_start(wqk_sb[:, 0], wqk_d[:, 0])
            warm = const.tile([128, 512], BF16)
            nc.vector.memset(warm, 0.0)
            for _ in range(16):
                wps = psb.tile([64, 512], F32, tag="acc", bufs=2, name="warm")
                nc.tensor.matmul(wps, warm[:, 0:64], warm,
                                 start=True, stop=True)
            xt0 = emit_xt_dma(0, sliced=True)
            nc.sync.dma_start(wqk_sb[:, 1], wqk_d[:, 1])
            nc.sync.dma_start(bq_sb, bq_d[:].rearrange("(t p) -> p t", p=128))
            for jt in range(2, 4):
                nc.sync.dma_start(wqk_sb[:, jt], wqk_d[:, jt])
            nc.sync.dma_start(wv_sb, wv_d[:])
            for jt in range(4, 8):
                nc.sync.dma_start(wqk_sb[:, jt], wqk_d[:, jt])
            pcs0 = a_pieces(0, xt0, acc_tag="S", acc_bufs=2)
            for piece in pcs0:
                piece()
            nc.sync.dma_start(wp_sb, wp_d[:])
            a_left = [0] * QC  # un-flushed A pieces per chunk

            def count_piece(piece, qc):
                def run():
                    a_left[qc] -= 1
                    piece()
                return run

            for qc in range(QC):
                q0 = qc * 512
                n_kt = 4 * (qc + 1)
                if qc + 1 < QC:
                    xt_n = emit_xt_dma(qc + 1)
                    pcs = a_pieces(qc + 1, xt_n)
                    a_left[qc + 1] = len(pcs)
                    pending.extend(count_piece(pc, qc + 1) for pc in pcs)
                # emission barrier: attention for qc depends on chunk qc's
                # Q/K/V writes being *emitted* (Tile tracks deps in trace
                # order); normally a no-op since pieces drain during qc-1.
                while a_left[qc] > 0:
                    flush_one()
                # rate-based filler spreading: drain the deferred queues
                # evenly over this chunk's flush slots so PE has work in
                # every exp-bound kt iteration, not just the first few.
                slots_total = PAIRS * max(1, n_kt + 1)
                slot_i = [0]
                flushed = [0]

                def flush_rate():
                    # linear spread: by slot i, i/slots of the (running)
                    # queue total should have drained
                    slot_i[0] += 1
                    q_total = flushed[0] + len(pending) + (
                        len(pending_late) if allow_late[0] else 0)
                    target = (q_total * slot_i[0]) // slots_total
                    while flushed[0] < target and (
                            pending or (pending_late and allow_late[0])):
                        flush_one()
                        flushed[0] += 1
                # per-chunk normalization staging (one DMA per chunk each;
                # the last chunk finishes per-pair so fins aren't end-bound)
                last_qc = (qc == QC - 1)
                allow_late[0] = last_qc
                stg_b = wk.tile([64, PAIRS, 512], BF16, tag="stgb", bufs=2)
                dn = wk.tile([65, PAIRS, 1024], F32, tag="dn", bufs=1)
                sums = wk.tile([2, PAIRS, 512], F32, tag="sums", bufs=1)
                rec_r = wk.tile([2, PAIRS, 512], mybir.dt.float32r,
                                tag="recr", bufs=2)

                fin_delay = list(fins_hold)  # prev chunk's fins: release
                del fins_hold[:]             # mid-pair, past their DMA chain

                def make_fin(p, q0, rec_r):
                    def fin():
                        bc = psb.tile([128, 512], F32, tag="acc", bufs=2,
                                      name="bc")
                        nc.tensor.matmul(bc, selab_r, rec_r[:, p, :],
                                         start=True, stop=True)
                        nc.vector.tensor_mul(y_sb[:, p, q0:q0 + 512],
                                             y_sb[:, p, q0:q0 + 512], bc)
                    return fin

                for p in range(PAIRS):
                    o_a = psb.tile([65, 512], F32, tag="O", bufs=2, name="o_a")
                    o_b = psb.tile([65, 512], F32, tag="O", bufs=2, name="o_b")
                    staged = {}

                    def emit_s(kt):
                        off = max(0, kt * 128 - q0)
                        # S for both heads in one 2-bank psum tile so one
                        # ScalarE exp covers both. fp8 DoubleRow: stationary
                        # K~ (hi, lo) planes, moving Q8 (dup planes).
                        s_ab = psb.tile([128, 1024], F32, tag="S", bufs=2,
                                        name="s_ab")
                        for h in range(2):
                            r0, r1 = h * 64, h * 64 + 64
                            lhs = kt_sb[r0:r1, p, :, kt * 128:(kt + 1) * 128]
                            cuts = ([(off, 256), (256, 512)] if off < 256
                                    else [(off, 512)])
                            for c0, c1 in cuts:
                                nc.tensor.matmul(
                                    s_ab[:, h * 512 + c0:h * 512 + c1],
                                    lhs,
                                    qt_sb[r0:r1, p, :, q0 + c0:q0 + c1],
                                    start=True, stop=True, perf_mode=DR)
                        staged[kt] = (s_ab, off)

                    def emit_exp_mask(kt):
                        s_ab, off = staged[kt]
                        p_ab = wk.tile([128, 1024], BF16, tag="P", bufs=10,
                                       name="p_ab")
                        s3 = s_ab.rearrange("p (c w) -> p c w", c=2)
                        p3 = p_ab.rearrange("p (c w) -> p c w", c=2)
                        nc.scalar.activation(
                            p3[:, :, off:512], s3[:, :, off:512],
                            mybir.ActivationFunctionType.Exp, scale=LN2)
                        if kt * 128 >= q0:  # causal diagonal block
                            # gpsimd: SBUF-only op on an otherwise idle
                            # engine, off DVE and off the exp->PV chain
                            nc.gpsimd.tensor_mul(
                                p3[:, :, off:off + 128],
                                p3[:, :, off:off + 128], triu2)
                        staged[kt] = (s_ab, off, p_ab)

                    def emit_pv(kt):
                        _, off, p_ab = staged.pop(kt)
                        first, last = (kt == 0), (kt == n_kt - 1)
                        nc.tensor.matmul(o_a[:, off:512],
                                         v_sb[:, kt, 2 * p, :],
                                         p_ab[:, off:512],
                                         start=first, stop=last)
                        nc.tensor.matmul(o_b[:, off:512],
                                         v_sb[:, kt, 2 * p + 1, :],
                                         p_ab[:, 512 + off:1024],
                                         start=first, stop=last)

                    # software pipeline: S(kt) and exp(kt-1)/mask(kt-1) go
                    # out first (ScalarE/DVE get max lead time), filler
                    # pieces next (PE is in-order: they must precede the
                    # PV that waits on exp), PV(kt-2) last -- the 2-step PV
                    # lag keeps the first PV of a pair (which waits for the
                    # previous pair's O-bank release through its staging
                    # copies) out of PE's in-order queue for ~2 kt.
                    LAG = 3
                    for kt in range(n_kt + LAG):
                        if kt < n_kt:
                            emit_s(kt)
                        if 1 <= kt <= n_kt:
                            emit_exp_mask(kt - 1)
                        if kt == min(11, n_kt) and fin_delay:
                            pending.extend(fin_delay)
                            del fin_delay[:]
                        if kt >= 2:
                            flush_rate()
                        if kt >= LAG:
                            emit_pv(kt - LAG)
                    # tail staging for this pair (head A lands aligned)
                    nc.vector.tensor_copy(y_sb[0:64, p, q0:q0 + 512],
                                          o_a[0:64, :])
                    if not (last_qc and p == PAIRS - 1):
                        # o_a's readers first so its PSUM bank frees one
                        # copy earlier for the next pair's accumulators
                        nc.vector.tensor_copy(dn[64:65, p, 0:512],
                                              o_a[64:65, :])
                    nc.vector.tensor_copy(stg_b[:, p, :], o_b[0:64, :])
                    if last_qc and p == PAIRS - 1:
                        # final tail: ScalarE is idle; also keep PE's clock
                        # ramped through the normalization chain with
                        # dependency-free dummy matmuls, so the fin/proj
                        # matmuls that follow don't run at the low p-state
                        nc.scalar.copy(dn[64:65, p, 0:512], o_a[64:65, :])
                        nc.scalar.copy(dn[64:65, p, 512:1024],
                                       o_b[64:65, :])
                        for _ in range(30):
                            wps = psb.tile([64, 512], F32, tag="acc",
                                           bufs=2, name="warm2")
                            nc.tensor.matmul(wps, warm[:, 0:64], warm,
                                             start=True, stop=True)
                    else:
                        nc.vector.tensor_copy(dn[64:65, p, 512:1024],
                                              o_b[64:65, :])
                    if last_qc:
                        for h in range(2):
                            nc.sync.dma_start(
                                sums[h:h + 1, p:p + 1, :],
                                dn[64:65, p:p + 1,
                                   h * 512:(h + 1) * 512])
                        nc.sync.dma_start(
                            y_sb[64:128, p:p + 1, q0:q0 + 512],
                            stg_b[:, p:p + 1, :])
                        rec = wk.tile([2, 1, 512], F32, tag="rec", bufs=2)
                        nc.vector.reciprocal_approx_fast(
                            rec, sums[:, p:p + 1, :])
                        nc.vector.tensor_copy(rec_r[:, p:p + 1, :], rec)
                        fin_delay.append(make_fin(p, q0, rec_r))
                if not last_qc:
                    # two partition-restructure DMAs + one reciprocal:
                    # sums[h, p, :] <- dn[64, p, h*512:(h+1)*512]
                    for h in range(2):
                        nc.sync.dma_start(sums[h:h + 1],
                                          dn[64:65, :, h * 512:(h + 1) * 512])
                    nc.sync.dma_start(y_sb[64:128, :, q0:q0 + 512], stg_b)
                    for p in range(PAIRS):
                        rec = wk.tile([2, 1, 512], F32, tag="rec", bufs=2)
                        nc.vector.reciprocal_approx_fast(
                            rec, sums[:, p:p + 1, :])
                        # f32r inputs must come from a rounding producer
                        nc.vector.tensor_copy(rec_r[:, p:p + 1, :], rec)
                        fins_hold.append(make_fin(p, q0, rec_r))
                pending.extend(fin_delay)
                del fin_delay[:]
                for tt in range(qc * 4, qc * 4 + 4):
                    for oc in range(2):
                        pending_late.append(make_proj_piece(
                            tt, oc, 0, split_dma=last_qc))
                        pending_late.append(make_proj_piece(
                            tt, oc, 1, split_dma=last_qc))
            flush_pending()
            if debug_taps:
                nc.sync.dma_start(dbg["y"][:], y_sb)
                nc.sync.dma_start(dbg["qt"][:], qt_sb)
                nc.sync.dma_start(dbg["kt"][:], kt_sb)
                nc.sync.dma_start(
                    dbg["v"][:, :], v_sb.rearrange("p a b c -> p (a b c)"))

    nc.compile()
    return nc


_NC_CACHE = {}


def _get_nc():
    if "nc" not in _NC_CACHE:
        _NC_CACHE["nc"] = build_nc()
    return _NC_CACHE["nc"]


def shard_inputs(x, W_attn, b_attn, W_proj):
    """Per-core input maps. Core c: batch c//2, head group c%2.

    b_attn handling (exact): b_k's S terms are constant along the softmax
    axis and cancel; b_q is applied on-device in Q's PSUM->SBUF copy; b_v
    shifts y by a constant so its c_proj image is added on the host (see
    kernel()). W_k is pre-scaled by log2e/8 so S scores are directly
    log2-softmax logits. All big tensors are pre-shuffled partition-major
    (the `p` axis below is the SBUF partition) so each SBUF resident fills
    with one (or a few) large-descriptor DMAs.
    """
    bf = ml_dtypes.bfloat16
    x = np.asarray(x, dtype=np.float32)
    W_attn = np.asarray(W_attn, dtype=np.float32)
    b_attn = np.asarray(b_attn, dtype=np.float32)
    W_proj = np.asarray(W_proj, dtype=np.float32)
    in_maps = []
    for c in range(N_CORES):
        b, hg = c // 2, c % 2
        qs, ks, vs = hg * J, C + hg * J, 2 * C + hg * J
        wqk = np.concatenate(
            [W_attn[:, qs:qs + J], W_attn[:, ks:ks + J] * K_SCALE], axis=1)
        # [C, 1024] -> [p, jt, ct, 128]: row ct*128+p, col jt*128+f
        wqk = np.ascontiguousarray(
            wqk.reshape(C_TILES, 128, 8, 128).transpose(1, 2, 0, 3)
        ).astype(bf)
        bq = np.ascontiguousarray(b_attn[qs:qs + J])
        # [C, J] -> [p, ct, J]
        wv = np.ascontiguousarray(
            W_attn[:, vs:vs + J].reshape(C_TILES, 128, J)
            .transpose(1, 0, 2)).astype(bf)
        # [J, C] -> [p, pair, C]
        wp = np.ascontiguousarray(
            W_proj[hg * J:(hg + 1) * J, :].reshape(PAIRS, 128, C)
            .transpose(1, 0, 2)).astype(bf)
        # [T, C] -> x^T [p, ct, T]
        xt = np.ascontiguousarray(
            x[b].T.reshape(C_TILES, 128, T).transpose(1, 0, 2)).astype(bf)
        in_maps.append({
            "xt": xt, "wqk": wqk, "bq": bq, "wv": wv, "wp": wp,
        })
    return in_maps


def kernel(x, W_attn, b_attn, W_proj, b_proj):
    nc = _get_nc()
    in_maps = shard_inputs(x, W_attn, b_attn, W_proj)
    res = run_bass_kernel_spmd(nc, in_maps, list(range(N_CORES)))
    b_attn = np.asarray(b_attn, dtype=np.float32)
    W_proj = np.asarray(W_proj, dtype=np.float32)
    b_proj = np.asarray(b_proj, dtype=np.float32)
    # host-side constant row: b_proj + (b_v @ W_proj)   (exact; see docstring)
    bias_row = b_proj + b_attn[2 * C:] @ W_proj
    outs = []
    for b in range(4):
        partial = (res.results[2 * b]["out"].astype(np.float32)
                   + res.results[2 * b + 1]["out"].astype(np.float32))
        outs.append(partial + bias_row[None, :])
    return np.stack(outs, axis=0)
